# revision 7
# baseline (speedup 1.0000x reference)
"""Trainium2 Bass kernel for nn_Cal_Div_Loss (conv-pyramid L1 loss).

Strategy
--------
The 3x3 all-ones stride-2 VALID conv ("edgesum") is linear, so the x- and
y-pyramids collapse into a single pyramid over d = x - y.  Per sample we
need sum(d) (for the 'last' column) and sum(|d_l|) at 5 pyramid levels
(512 -> 255 -> 127 -> 63 -> 31).  The tiny cross-batch 'fuhao' sign logic
and the final mean are O(B*6) and run on the host.

Sharding: data-parallel over batch, 64 samples / 8 cores = 8 samples/core.
Per core 16 MiB of input -> DMA-bound at ~358 GB/s (~47 us) — the target.

Per level, edgesum(d) = R @ d @ R^T (R = banded ones, window 3 stride 2):
  - column-window sum (d @ R^T) = two strided tensor_tensor adds,
    SBUF -> SBUF, on DVE (and GPSIMD for half the samples at level 0)
  - row-window sum (R @ .) = matmuls with the banded R^T chunks as the
    stationary operand; samples are blocked so one LDWEIGHTS serves
    several matmuls
  - deep levels (1-3) write all 8 samples' matmul outputs into one
    8-bank PSUM tile (sample index = bank), evacuated by a single
    batched ACT copy; column sums and |.| stats then run batched
  - d = x - y is fused with the signed sum via DVE scalar_tensor_tensor
    accum_out; |d| level-0 stats come from ACT Abs with accum_out
"""

import sys

if "/opt/trn_rl_repo" not in sys.path:
    sys.path.insert(0, "/opt/trn_rl_repo")

import numpy as np

# ---------------------------------------------------------------- constants
B = 64          # full batch
NCORES = 8
S = B // NCORES  # samples per core
P = 128
N0, N1, N2, N3, N4 = 512, 255, 127, 63, 31
G0 = 4          # 128-row chunks at level 0
LAYER_NUM = 4

# samples whose level-0 column sum runs on GPSIMD instead of DVE
GPSIMD_COLSUM_SAMPLES = (1, 3, 5, 7)

# stats tile columns: [0:8] sd, [8:16] sa0, [16:24] sa1 rows 0..127,
# [24:32] sa1 rows 128..254, [32:40] sa2, [40:48] sa3, [48:56] sa4
STATS_COLS = 64

_CACHE = {}


def _banded(n_out, n_in, pad_to=None):
    """R^T for the window-3 stride-2 row sum: [n_in, n_out] fp32."""
    r = np.zeros((n_out, n_in), dtype=np.float32)
    for i in range(n_out):
        r[i, 2 * i : 2 * i + 3] = 1.0
    bt = np.ascontiguousarray(r.T)
    if pad_to is not None and pad_to > n_in:
        bt = np.concatenate(
            [bt, np.zeros((pad_to - n_in, n_out), dtype=np.float32)], axis=0
        )
    return bt


def _colsum(nc, eng, out, src):
    """out = src[..., 0::2] + src[..., 1::2] + src[..., 2::2] (win 3 stride 2).

    src must be in SBUF."""
    n_out = out.shape[-1]
    sl = [slice(None)] * (len(src.shape) - 1)
    e0 = src[tuple(sl + [slice(0, 2 * n_out - 1, 2)])]
    e1 = src[tuple(sl + [slice(1, 2 * n_out, 2)])]
    e2 = src[tuple(sl + [slice(2, 2 * n_out + 1, 2)])]
    eng.tensor_add(out=out, in0=e0, in1=e1)
    eng.tensor_add(out=out, in0=out, in1=e2)


def _build_nc():
    from contextlib import ExitStack

    import concourse.bacc as bacc
    import concourse.mybir as mybir
    import concourse.tile as tile

    f32 = mybir.dt.float32
    SUB = mybir.AluOpType.subtract
    ADD = mybir.AluOpType.add
    AX = mybir.AxisListType.X
    AF = mybir.ActivationFunctionType

    nc = bacc.Bacc("TRN2", target_bir_lowering=False, debug=False)
    xs = nc.dram_tensor("xs", [S, 512, 512], f32, kind="ExternalInput").ap()
    ys = nc.dram_tensor("ys", [S, 512, 512], f32, kind="ExternalInput").ap()
    bt0 = nc.dram_tensor("bt0", [512, N1], f32, kind="ExternalInput").ap()
    bt1 = nc.dram_tensor("bt1", [256, N2], f32, kind="ExternalInput").ap()
    bt2 = nc.dram_tensor("bt2", [N2, N3], f32, kind="ExternalInput").ap()
    bt3 = nc.dram_tensor("bt3", [N3, N4], f32, kind="ExternalInput").ap()
    stats_out = nc.dram_tensor(
        "stats", [P, STATS_COLS], f32, kind="ExternalOutput"
    ).ap()

    with tile.TileContext(nc) as tc, ExitStack() as ctx:
        singles = ctx.enter_context(tc.tile_pool(name="singles", bufs=1))
        xy = ctx.enter_context(tc.tile_pool(name="xy", bufs=3))
        dpool = ctx.enter_context(tc.tile_pool(name="d", bufs=2))
        vpool = ctx.enter_context(tc.tile_pool(name="v", bufs=4))
        scr = ctx.enter_context(tc.tile_pool(name="scr", bufs=1))

        # banded-ones constants (stationary matmul operands)
        bt0_sb = singles.tile([P, G0, N1], f32)
        nc.sync.dma_start(out=bt0_sb, in_=bt0.rearrange("(g p) i -> p g i", p=P))
        bt1_sb = singles.tile([P, 2, N2], f32)
        nc.sync.dma_start(out=bt1_sb, in_=bt1.rearrange("(g p) i -> p g i", p=P))
        bt2_sb = singles.tile([N2, N3], f32)
        nc.sync.dma_start(out=bt2_sb, in_=bt2)
        bt3_sb = singles.tile([N3, N4], f32)
        nc.sync.dma_start(out=bt3_sb, in_=bt3)

        # persistent per-level images, batched over samples in the free dim
        d1a = singles.tile([P, S, N1], f32)    # d1 rows 0..127
        d1b = singles.tile([127, S, N1], f32)  # d1 rows 128..254
        v1a = singles.tile([P, S, N2], f32)
        v1b = singles.tile([127, S, N2], f32)
        d2A = singles.tile([N2, S, N2], f32)
        v2A = singles.tile([N2, S, N3], f32)
        d3A = singles.tile([N3, S, N3], f32)
        v3A = singles.tile([N3, S, N4], f32)
        d4A = singles.tile([N4, S, N4], f32)
        stats = singles.tile([P, STATS_COLS], f32)
        nc.vector.memset(stats, 0.0)

        # ------------- phase 0: per-sample level-0 work, 2-sample blocks ---
        with tc.tile_pool(name="pd1", bufs=4, space="PSUM") as pd1:
            v0s = {}
            for s in range(S):
                xt = xy.tile([P, G0, N0], f32, tag="xt")
                yt = xy.tile([P, G0, N0], f32, tag="yt")
                nc.sync.dma_start(
                    out=xt, in_=xs[s].rearrange("(g p) c -> p g c", p=P)
                )
                nc.sync.dma_start(
                    out=yt, in_=ys[s].rearrange("(g p) c -> p g c", p=P)
                )

                dt = dpool.tile([P, G0, N0], f32, tag="dt")
                nc.vector.scalar_tensor_tensor(
                    out=dt, in0=xt, scalar=0.0, in1=yt,
                    op0=ADD, op1=SUB, accum_out=stats[:, s : s + 1],
                )

                ascr = scr.tile([P, G0, N0], f32, tag="ascr")
                nc.scalar.activation(
                    out=ascr, in_=dt, func=AF.Abs,
                    accum_out=stats[:, 8 + s : 9 + s],
                )

                # col-window sum (SBUF->SBUF): v0 [P, G0, N1]
                v0 = vpool.tile([P, G0, N1], f32, tag="v0")
                eng = (
                    nc.gpsimd if s in GPSIMD_COLSUM_SAMPLES else nc.vector
                )
                _colsum(nc, eng, v0, dt)
                v0s[s] = v0

                # after each odd sample: banded matmuls for the pair,
                # sharing each LDWEIGHTS across both samples
                if s % 2 == 1:
                    pair = (s - 1, s)
                    w = {
                        (sp, m): pd1.tile(
                            [P, N1], f32, tag="pd1", name=f"w_{sp}_{m}"
                        )
                        for sp in pair
                        for m in (0, 1)
                    }
                    for m, gs in ((0, (0, 1, 2)), (1, (2, 3))):
                        mp = 128 if m == 0 else 127
                        for j, g in enumerate(gs):
                            for sp in pair:
                                nc.tensor.matmul(
                                    w[(sp, m)][:mp, :],
                                    bt0_sb[:, g, m * 128 : m * 128 + mp],
                                    v0s[sp][:, g, :],
                                    start=(j == 0),
                                    stop=(j == len(gs) - 1),
                                )
                    for sp in pair:
                        nc.scalar.copy(
                            out=d1a[:, sp, :], in_=w[(sp, 0)][:, :]
                        )
                        nc.scalar.copy(
                            out=d1b[:, sp, :], in_=w[(sp, 1)][:127, :]
                        )
                    v0s = {}

        # level-1 |.| stats can start as soon as d1 is complete
        nc.vector.tensor_reduce(
            out=stats[:, 16:24], in_=d1a, axis=AX, op=ADD,
            apply_absolute_value=True,
        )
        nc.vector.tensor_reduce(
            out=stats[0:127, 24:32], in_=d1b, axis=AX, op=ADD,
            apply_absolute_value=True,
        )

        # ------------- phase 1: level 1 (batched colsum + PE) --------------
        _colsum(nc, nc.vector, v1a, d1a)
        _colsum(nc, nc.gpsimd, v1b, d1b)
        with tc.tile_pool(name="pbig", bufs=1, space="PSUM") as pbig:
            wb2 = pbig.tile([N2, S, 512], f32, tag="pbig")
            for g, (bt_sl, v_sl) in enumerate(
                ((bt1_sb[:, 0, :], v1a), (bt1_sb[0:127, 1, :], v1b))
            ):
                for s in range(S):
                    nc.tensor.matmul(
                        wb2[:, s, 0:N2],
                        bt_sl,
                        v_sl[:, s, :],
                        start=(g == 0),
                        stop=(g == 1),
                    )
            nc.scalar.copy(out=d2A, in_=wb2[:, :, 0:N2])

            nc.vector.tensor_reduce(
                out=stats[0:127, 32:40], in_=d2A, axis=AX, op=ADD,
                apply_absolute_value=True,
            )

            # ------------- phase 2: level 2 --------------------------------
            _colsum(nc, nc.vector, v2A, d2A)
            wb3 = pbig.tile([N3, S, 512], f32, tag="pbig")
            for s in range(S):
                nc.tensor.matmul(
                    wb3[:, s, 0:N3], bt2_sb, v2A[:, s, :], start=True, stop=True
                )
            nc.scalar.copy(out=d3A, in_=wb3[:, :, 0:N3])

            nc.vector.tensor_reduce(
                out=stats[0:63, 40:48], in_=d3A, axis=AX, op=ADD,
                apply_absolute_value=True,
            )

            # ------------- phase 3: level 3 --------------------------------
            _colsum(nc, nc.vector, v3A, d3A)
            wb4 = pbig.tile([N4, S, 512], f32, tag="pbig")
            for s in range(S):
                nc.tensor.matmul(
                    wb4[:, s, 0:N4], bt3_sb, v3A[:, s, :], start=True, stop=True
                )
            nc.scalar.copy(out=d4A, in_=wb4[:, :, 0:N4])

        nc.vector.tensor_reduce(
            out=stats[0:31, 48:56], in_=d4A, axis=AX, op=ADD,
            apply_absolute_value=True,
        )

        nc.sync.dma_start(out=stats_out, in_=stats)

    nc.finalize()
    return nc


def _get_nc():
    if "nc" not in _CACHE:
        _CACHE["nc"] = _build_nc()
    return _CACHE["nc"]


def _run_on_hw(x, y, trace=False):
    """x, y: [64, 512, 512] fp32 numpy. Returns list of 8 stats arrays."""
    from concourse.bass_utils import run_bass_kernel_spmd

    nc = _get_nc()
    bt0 = _banded(N1, 512)
    bt1 = _banded(N2, N1, pad_to=256)
    bt2 = _banded(N3, N2)
    bt3 = _banded(N4, N3)

    in_maps = []
    for c in range(NCORES):
        in_maps.append(
            {
                "xs": np.ascontiguousarray(x[c * S : (c + 1) * S]),
                "ys": np.ascontiguousarray(y[c * S : (c + 1) * S]),
                "bt0": bt0,
                "bt1": bt1,
                "bt2": bt2,
                "bt3": bt3,
            }
        )

    res = run_bass_kernel_spmd(
        nc, in_maps, core_ids=list(range(NCORES)), trace=trace
    )
    _CACHE["last_results"] = res
    return [r["stats"] for r in res.results]


def kernel(x, y, alpha, _trace=False):
    x = np.ascontiguousarray(np.asarray(x, dtype=np.float32).reshape(B, 512, 512))
    y = np.ascontiguousarray(np.asarray(y, dtype=np.float32).reshape(B, 512, 512))
    alpha = np.asarray(alpha, dtype=np.float32)

    stats_list = _run_on_hw(x, y, trace=_trace)

    sd = np.empty(B, np.float64)
    sa = np.empty((B, 5), np.float64)
    for c in range(NCORES):
        st = stats_list[c].astype(np.float64)
        for s in range(S):
            b = c * S + s
            sd[b] = st[:, s].sum()
            sa[b, 0] = st[:, 8 + s].sum()
            sa[b, 1] = st[:, 16 + s].sum() + st[0:127, 24 + s].sum()
            sa[b, 2] = st[0:127, 32 + s].sum()
            sa[b, 3] = st[0:63, 40 + s].sum()
            sa[b, 4] = st[0:31, 48 + s].sum()

    counts = np.array(
        [N0 * N0, N1 * N1, N2 * N2, N3 * N3, N4 * N4], np.float64
    )
    l1 = sa / counts  # [B, 5]
    last = np.abs(sd) * float(LAYER_NUM + 1)  # [B]

    # faithful 'fuhao' replication (matches reference.py exactly)
    k_layer = (alpha * np.float32(LAYER_NUM + 2)).astype(np.int32)  # [B]
    trig = k_layer <= LAYER_NUM
    triggered_before = np.concatenate(
        [np.zeros(1, bool), np.cumsum(trig)[:-1] > 0]
    )
    i_idx = np.arange(LAYER_NUM + 1)
    sign = np.where(
        triggered_before[:, None] | (i_idx[None, :] >= k_layer[:, None]),
        1.0,
        -1.0,
    )

    loss_tensor = np.concatenate([l1 * sign, last[:, None]], axis=1)
    return np.float32(loss_tensor.mean())


# revision 8
# speedup vs baseline: 1.3613x; 1.3613x over previous
"""Trainium2 Bass kernel for nn_Cal_Div_Loss (conv-pyramid L1 loss).

Strategy
--------
The 3x3 all-ones stride-2 VALID conv ("edgesum") is linear, so the x- and
y-pyramids collapse into a single pyramid over d = x - y.  Per sample we
need sum(d) (for the 'last' column) and sum(|d_l|) at 5 pyramid levels
(512 -> 255 -> 127 -> 63 -> 31).  The tiny cross-batch 'fuhao' sign logic
and the final mean are O(B*6) and run on the host.

Sharding: data-parallel over batch, 64 samples / 8 cores = 8 samples/core.
Per core 16 MiB of input -> DMA-bound at ~358 GB/s (~47 us) — the target.

Per level, edgesum(d) = R @ d @ R^T (R = banded ones, window 3 stride 2):
  - column-window sum (d @ R^T) = two strided tensor_tensor adds on DVE,
    SBUF -> SBUF; the result is written in bf16 (exact-1.0 banded weights,
    fp32 PSUM accumulation — only the matmul *inputs* are rounded, and the
    rounding averages out in the |.| sums)
  - row-window sum (R @ .) = bf16 matmuls with the banded R^T chunks as
    the stationary operand (4x faster than fp32 on the PE)
  - deep levels (1-3) write all 8 samples' matmul outputs into one
    8-bank PSUM tile (sample index = bank), evacuated by a single
    batched ACT copy; column sums and |.| stats then run batched
  - d = x - y is fused with the signed sum via DVE scalar_tensor_tensor
    accum_out; |d| level-0 stats come from ACT Abs with accum_out
"""

import sys

if "/opt/trn_rl_repo" not in sys.path:
    sys.path.insert(0, "/opt/trn_rl_repo")

import numpy as np

# ---------------------------------------------------------------- constants
B = 64          # full batch
NCORES = 8
S = B // NCORES  # samples per core
P = 128
N0, N1, N2, N3, N4 = 512, 255, 127, 63, 31
G0 = 4          # 128-row chunks at level 0
LAYER_NUM = 4

# stats tile columns: [0:8] sd, [8:16] sa0, [16:24] sa1 rows 0..127,
# [24:32] sa1 rows 128..254, [32:40] sa2, [40:48] sa3, [48:56] sa4
STATS_COLS = 64

_CACHE = {}


def _banded(n_out, n_in, pad_to=None):
    """R^T for the window-3 stride-2 row sum: [n_in, n_out] bf16."""
    import ml_dtypes

    r = np.zeros((n_out, n_in), dtype=np.float32)
    for i in range(n_out):
        r[i, 2 * i : 2 * i + 3] = 1.0
    bt = np.ascontiguousarray(r.T)
    if pad_to is not None and pad_to > n_in:
        bt = np.concatenate(
            [bt, np.zeros((pad_to - n_in, n_out), dtype=np.float32)], axis=0
        )
    return bt.astype(ml_dtypes.bfloat16)


def _build_nc():
    from contextlib import ExitStack

    import concourse.bacc as bacc
    import concourse.mybir as mybir
    import concourse.tile as tile

    f32 = mybir.dt.float32
    bf16 = mybir.dt.bfloat16
    SUB = mybir.AluOpType.subtract
    ADD = mybir.AluOpType.add
    AX = mybir.AxisListType.X
    AF = mybir.ActivationFunctionType

    nc = bacc.Bacc("TRN2", target_bir_lowering=False, debug=False)
    xs = nc.dram_tensor("xs", [S, 512, 512], f32, kind="ExternalInput").ap()
    ys = nc.dram_tensor("ys", [S, 512, 512], f32, kind="ExternalInput").ap()
    bt0 = nc.dram_tensor("bt0", [512, N1], bf16, kind="ExternalInput").ap()
    bt1 = nc.dram_tensor("bt1", [256, N2], bf16, kind="ExternalInput").ap()
    bt2 = nc.dram_tensor("bt2", [N2, N3], bf16, kind="ExternalInput").ap()
    bt3 = nc.dram_tensor("bt3", [N3, N4], bf16, kind="ExternalInput").ap()
    stats_out = nc.dram_tensor(
        "stats", [P, STATS_COLS], f32, kind="ExternalOutput"
    ).ap()

    def colsum(out, src, u):
        """out = src[...,0::2] + src[...,1::2] + src[...,2::2].

        Two DVE adds; `u` is an fp32 scratch so the bf16 cast happens only
        on the final write."""
        n_out = out.shape[-1]
        sl = [slice(None)] * (len(src.shape) - 1)
        e0 = src[tuple(sl + [slice(0, 2 * n_out - 1, 2)])]
        e1 = src[tuple(sl + [slice(1, 2 * n_out, 2)])]
        e2 = src[tuple(sl + [slice(2, 2 * n_out + 1, 2)])]
        nc.vector.tensor_add(out=u, in0=e0, in1=e1)
        nc.vector.tensor_add(out=out, in0=u, in1=e2)

    with tile.TileContext(nc) as tc, ExitStack() as ctx:
        singles = ctx.enter_context(tc.tile_pool(name="singles", bufs=1))
        xy = ctx.enter_context(tc.tile_pool(name="xy", bufs=3))
        dpool = ctx.enter_context(tc.tile_pool(name="d", bufs=2))
        vpool = ctx.enter_context(tc.tile_pool(name="v", bufs=4))
        upool = ctx.enter_context(tc.tile_pool(name="u", bufs=2))
        scr = ctx.enter_context(tc.tile_pool(name="scr", bufs=1))

        # banded-ones constants (stationary matmul operands), bf16
        bt0_sb = singles.tile([P, G0, N1], bf16)
        nc.sync.dma_start(out=bt0_sb, in_=bt0.rearrange("(g p) i -> p g i", p=P))
        bt1_sb = singles.tile([P, 2, N2], bf16)
        nc.sync.dma_start(out=bt1_sb, in_=bt1.rearrange("(g p) i -> p g i", p=P))
        bt2_sb = singles.tile([N2, N3], bf16)
        nc.sync.dma_start(out=bt2_sb, in_=bt2)
        bt3_sb = singles.tile([N3, N4], bf16)
        nc.sync.dma_start(out=bt3_sb, in_=bt3)

        # persistent per-level images (fp32, padded strides) and their
        # column sums (bf16, matmul moving operands)
        d1a = singles.tile([P, S, 256], f32)    # d1 rows 0..127 (255 used)
        d1b = singles.tile([127, S, 256], f32)  # d1 rows 128..254
        v1a = singles.tile([P, S, 128], bf16)   # 127 used
        v1b = singles.tile([127, S, 128], bf16)
        d2A = singles.tile([N2, S, 128], f32)
        v2A = singles.tile([N2, S, 64], bf16)
        d3A = singles.tile([N3, S, 64], f32)
        v3A = singles.tile([N3, S, 32], bf16)
        d4A = singles.tile([N4, S, 32], f32)
        stats = singles.tile([P, STATS_COLS], f32)
        nc.vector.memset(stats, 0.0)

        # ------------- phase 0: per-sample level-0 work, 2-sample blocks ---
        with tc.tile_pool(name="pd1", bufs=4, space="PSUM") as pd1:
            v0s = {}
            for s in range(S):
                xt = xy.tile([P, G0, N0], f32, tag="xt")
                yt = xy.tile([P, G0, N0], f32, tag="yt")
                nc.sync.dma_start(
                    out=xt, in_=xs[s].rearrange("(g p) c -> p g c", p=P)
                )
                nc.sync.dma_start(
                    out=yt, in_=ys[s].rearrange("(g p) c -> p g c", p=P)
                )

                dt = dpool.tile([P, G0, N0], f32, tag="dt")
                nc.vector.scalar_tensor_tensor(
                    out=dt, in0=xt, scalar=0.0, in1=yt,
                    op0=ADD, op1=SUB, accum_out=stats[:, s : s + 1],
                )

                ascr = scr.tile([P, G0, N0], f32, tag="ascr")
                nc.scalar.activation(
                    out=ascr, in_=dt, func=AF.Abs,
                    accum_out=stats[:, 8 + s : 9 + s],
                )

                # col-window sum: v0 [P, G0, N1] bf16
                v0 = vpool.tile([P, G0, N1], bf16, tag="v0")
                u0 = upool.tile([P, G0, N1], f32, tag="u0")
                colsum(v0, dt, u0)
                v0s[s] = v0

                # after each odd sample: banded matmuls for the pair,
                # letting consecutive matmuls share each stationary chunk
                if s % 2 == 1:
                    pair = (s - 1, s)
                    w = {
                        (sp, m): pd1.tile(
                            [P, N1], f32, tag="pd1", name=f"w_{sp}_{m}"
                        )
                        for sp in pair
                        for m in (0, 1)
                    }
                    for m, gs in ((0, (0, 1, 2)), (1, (2, 3))):
                        mp = 128 if m == 0 else 127
                        for j, g in enumerate(gs):
                            for sp in pair:
                                nc.tensor.matmul(
                                    w[(sp, m)][:mp, :],
                                    bt0_sb[:, g, m * 128 : m * 128 + mp],
                                    v0s[sp][:, g, :],
                                    start=(j == 0),
                                    stop=(j == len(gs) - 1),
                                )
                    for sp in pair:
                        nc.scalar.copy(
                            out=d1a[:, sp, 0:N1], in_=w[(sp, 0)][:, :]
                        )
                        nc.scalar.copy(
                            out=d1b[:, sp, 0:N1], in_=w[(sp, 1)][:127, :]
                        )
                    v0s = {}

        # level-1 |.| stats can start as soon as d1 is complete
        nc.vector.tensor_reduce(
            out=stats[:, 16:24], in_=d1a[:, :, 0:N1], axis=AX, op=ADD,
            apply_absolute_value=True,
        )
        nc.vector.tensor_reduce(
            out=stats[0:127, 24:32], in_=d1b[:, :, 0:N1], axis=AX, op=ADD,
            apply_absolute_value=True,
        )

        # ------------- phase 1: level 1 (batched colsum + PE) --------------
        u1a = upool.tile([P, S, N2], f32, tag="u1", name="u1a")
        colsum(v1a[:, :, 0:N2], d1a[:, :, 0:N1], u1a)
        u1b = upool.tile([127, S, N2], f32, tag="u1", name="u1b")
        colsum(v1b[:, :, 0:N2], d1b[:, :, 0:N1], u1b)
        with tc.tile_pool(name="pbig", bufs=1, space="PSUM") as pbig:
            wb2 = pbig.tile([N2, S, 512], f32, tag="pbig")
            for g, (bt_sl, v_sl) in enumerate(
                ((bt1_sb[:, 0, :], v1a), (bt1_sb[0:127, 1, :], v1b))
            ):
                for s in range(S):
                    nc.tensor.matmul(
                        wb2[:, s, 0:N2],
                        bt_sl,
                        v_sl[:, s, 0:N2],
                        start=(g == 0),
                        stop=(g == 1),
                    )
            nc.scalar.copy(out=d2A[:, :, 0:N2], in_=wb2[:, :, 0:N2])

            nc.vector.tensor_reduce(
                out=stats[0:127, 32:40], in_=d2A[:, :, 0:N2], axis=AX, op=ADD,
                apply_absolute_value=True,
            )

            # ------------- phase 2: level 2 --------------------------------
            u2 = upool.tile([N2, S, N3], f32, tag="u2", name="u2")
            colsum(v2A[:, :, 0:N3], d2A[:, :, 0:N2], u2)
            wb3 = pbig.tile([N3, S, 512], f32, tag="pbig")
            for s in range(S):
                nc.tensor.matmul(
                    wb3[:, s, 0:N3], bt2_sb, v2A[:, s, 0:N3],
                    start=True, stop=True,
                )
            nc.scalar.copy(out=d3A[:, :, 0:N3], in_=wb3[:, :, 0:N3])

            nc.vector.tensor_reduce(
                out=stats[0:63, 40:48], in_=d3A[:, :, 0:N3], axis=AX, op=ADD,
                apply_absolute_value=True,
            )

            # ------------- phase 3: level 3 --------------------------------
            u3 = upool.tile([N3, S, N4], f32, tag="u3", name="u3")
            colsum(v3A[:, :, 0:N4], d3A[:, :, 0:N3], u3)
            wb4 = pbig.tile([N4, S, 512], f32, tag="pbig")
            for s in range(S):
                nc.tensor.matmul(
                    wb4[:, s, 0:N4], bt3_sb, v3A[:, s, 0:N4],
                    start=True, stop=True,
                )
            nc.scalar.copy(out=d4A[:, :, 0:N4], in_=wb4[:, :, 0:N4])

        nc.vector.tensor_reduce(
            out=stats[0:31, 48:56], in_=d4A[:, :, 0:N4], axis=AX, op=ADD,
            apply_absolute_value=True,
        )

        nc.sync.dma_start(out=stats_out, in_=stats)

    nc.finalize()
    return nc


def _get_nc():
    if "nc" not in _CACHE:
        _CACHE["nc"] = _build_nc()
    return _CACHE["nc"]


def _run_on_hw(x, y, trace=False):
    """x, y: [64, 512, 512] fp32 numpy. Returns list of 8 stats arrays."""
    from concourse.bass_utils import run_bass_kernel_spmd

    nc = _get_nc()
    bt0 = _banded(N1, 512)
    bt1 = _banded(N2, N1, pad_to=256)
    bt2 = _banded(N3, N2)
    bt3 = _banded(N4, N3)

    in_maps = []
    for c in range(NCORES):
        in_maps.append(
            {
                "xs": np.ascontiguousarray(x[c * S : (c + 1) * S]),
                "ys": np.ascontiguousarray(y[c * S : (c + 1) * S]),
                "bt0": bt0,
                "bt1": bt1,
                "bt2": bt2,
                "bt3": bt3,
            }
        )

    res = run_bass_kernel_spmd(
        nc, in_maps, core_ids=list(range(NCORES)), trace=trace
    )
    _CACHE["last_results"] = res
    return [r["stats"] for r in res.results]


def kernel(x, y, alpha, _trace=False):
    x = np.ascontiguousarray(np.asarray(x, dtype=np.float32).reshape(B, 512, 512))
    y = np.ascontiguousarray(np.asarray(y, dtype=np.float32).reshape(B, 512, 512))
    alpha = np.asarray(alpha, dtype=np.float32)

    stats_list = _run_on_hw(x, y, trace=_trace)

    sd = np.empty(B, np.float64)
    sa = np.empty((B, 5), np.float64)
    for c in range(NCORES):
        st = stats_list[c].astype(np.float64)
        for s in range(S):
            b = c * S + s
            sd[b] = st[:, s].sum()
            sa[b, 0] = st[:, 8 + s].sum()
            sa[b, 1] = st[:, 16 + s].sum() + st[0:127, 24 + s].sum()
            sa[b, 2] = st[0:127, 32 + s].sum()
            sa[b, 3] = st[0:63, 40 + s].sum()
            sa[b, 4] = st[0:31, 48 + s].sum()

    counts = np.array(
        [N0 * N0, N1 * N1, N2 * N2, N3 * N3, N4 * N4], np.float64
    )
    l1 = sa / counts  # [B, 5]
    last = np.abs(sd) * float(LAYER_NUM + 1)  # [B]

    # faithful 'fuhao' replication (matches reference.py exactly)
    k_layer = (alpha * np.float32(LAYER_NUM + 2)).astype(np.int32)  # [B]
    trig = k_layer <= LAYER_NUM
    triggered_before = np.concatenate(
        [np.zeros(1, bool), np.cumsum(trig)[:-1] > 0]
    )
    i_idx = np.arange(LAYER_NUM + 1)
    sign = np.where(
        triggered_before[:, None] | (i_idx[None, :] >= k_layer[:, None]),
        1.0,
        -1.0,
    )

    loss_tensor = np.concatenate([l1 * sign, last[:, None]], axis=1)
    return np.float32(loss_tensor.mean())
